# revision 17
# baseline (speedup 1.0000x reference)
"""AddSpatialInfo: out = concat([img_feat, coord_grid], axis=1).

img_feat [64, 2048, 14, 14] f32 -> out [64, 2050, 14, 14] f32.

Data-parallel over batch: 8 NeuronCores x 8 batches each. Per core this
is a pure DMA problem: copy the feature shard into the channel-strided
output and write a tiny NEFF-embedded coord constant into the last two
channels of each batch.

Performance notes (measured on TRN2 via neuron-profile):
- SDMA engine assignment is `outer AP dim index % 16`, so each per-batch
  copy is forced to a wide outer dim (128 descriptors of 12.5 KB) to
  engage all 16 engines; per-batch DMAs keep all engines inside one
  contiguous region at a time, which HBM likes (~300+ GB/s payload/core,
  chip-level HBM equilibrium).
- One DMA per batch beats one whole-shard DMA (engine/address locality)
  and beats splitting across both HWDGE rings (queue round-robin breaks
  locality).
- The stock Bass init emits an all-engine barrier my single-engine
  program doesn't need; _LeanBass skips it and instructions are emitted
  without a Block so there is no end-of-block barrier either (the final
  semaphore wait on the issuing engine is what gates completion).
"""

import numpy as np

import concourse.bass as bass
import concourse.mybir as mybir
from concourse.bass_utils import run_bass_kernel_spmd

B, C, H, W = 64, 2048, 14, 14
HW = H * W
N_CORES = 8
BPC = B // N_CORES  # batches per core
# Per-batch copy (401408 f32 = 1.6 MB) splits into 12.5 KB descriptors;
# outer dim count 128 % 16 == 0 keeps all 16 SDMA engines evenly loaded.
DESC_BYTES = 12544


class _LeanBass(bass.Bass):
    """Bass without the init-time all-engine barrier (single-engine
    program; nothing to order against)."""

    def all_engine_barrier(self, **kw):
        pass


def _coord_block() -> np.ndarray:
    # Match reference op-for-op in f32: x[j] = j*2/W - 1, y[i] = i*2/H - 1.
    xs = np.arange(W, dtype=np.float32) * np.float32(2.0) / np.float32(W) - np.float32(1.0)
    ys = np.arange(H, dtype=np.float32) * np.float32(2.0) / np.float32(H) - np.float32(1.0)
    x_ch = np.broadcast_to(xs[None, :], (H, W))
    y_ch = np.broadcast_to(ys[:, None], (H, W))
    coord = np.stack([x_ch, y_ch], axis=0)  # [2, H, W]
    return np.broadcast_to(coord[None], (BPC, 2, H, W)).reshape(BPC, 2, HW).copy()


def _build() -> bass.Bass:
    nc = _LeanBass()
    img = nc.declare_dram_parameter(
        "img_feat", [BPC, C, HW], mybir.dt.float32, isOutput=False
    )
    out = nc.declare_dram_parameter(
        "out", [BPC, C + 2, HW], mybir.dt.float32, isOutput=True
    )
    coord = nc.inline_tensor(_coord_block(), name="coord")
    dma_sem = nc.alloc_semaphore("dma_sem")

    sync = nc.sync
    for b in range(BPC):
        sync.dma_start(
            out=out[b, 0:C, :], in_=img[b], max_dma_last_dim=DESC_BYTES + 1
        ).then_inc(dma_sem, 16)
    sync.dma_start(out=out[:, C : C + 2, :], in_=coord[:]).then_inc(dma_sem, 16)
    # 9 DMAs (8 batch copies + coord), each +16 on completion.
    sync.wait_ge(dma_sem, 16 * (BPC + 1))
    return nc


def _run(img_feat: np.ndarray, **spmd_kwargs):
    """Run on 8 cores; returns (full_output, BassKernelResults)."""
    img_feat = np.ascontiguousarray(np.asarray(img_feat, dtype=np.float32))
    nc = _build()
    in_maps = [
        {"img_feat": img_feat[i * BPC : (i + 1) * BPC].reshape(BPC, C, HW)}
        for i in range(N_CORES)
    ]
    res = run_bass_kernel_spmd(nc, in_maps, core_ids=list(range(N_CORES)), **spmd_kwargs)
    out = np.concatenate(
        [
            np.asarray(res.results[i]["out"]).reshape(BPC, C + 2, H, W)
            for i in range(N_CORES)
        ],
        axis=0,
    )
    return out, res


def kernel(img_feat: np.ndarray) -> np.ndarray:
    out, _ = _run(img_feat)
    return out
